# revision 18
# baseline (speedup 1.0000x reference)
"""Trainium2 Bass kernel v2: single-head causal attention
(B=4, T=4096, C=2048, H=128), zero collectives.

    q = x @ Wq; k = x @ Wk; v = x @ Wv        (per batch element)
    out = softmax(causal(q k^T * C**-0.5)) @ v

Sharding: two cores per batch element (8 cores, B=4). Within a batch the
4096 q rows are split between the pair by 128-row-block parity (core p
owns blocks p, p+2, ...), which balances the causal work. Unlike v1,
each core projects k/v for the FULL 4096 rows from the full xT (the
extra ~4.3 GFLOP of redundant projection removes both AllGathers and all
cross-core synchronization).

The host permutes xT's 128-column blocks into "pair-local" order (own
parity block i at position 2i, the sibling block at 2i+1), so one
parity-free program serves all cores: q projects from even positions,
the causal band rule in position terms is dd==2c -> tri mask,
dd==2c+1 -> per-core mask (ones for p=1, zeros for p=0), dd>2c+1 ->
zeros.

Projections are weight-stationary (Wk/Wv/Wq chunks as PE weights, xT
chunks as the N=512 mover): kT and vT come out in [H, T] layout with
only 16 LDWEIGHTS each. V is then flipped to [k, h] blocks with 32
PE-mode transposes. Attention runs scores-transposed: S^T = kT_blk^T @
qT, P = exp(S^T * scale) on ACT, masking on the DVE (diagonal band
only), then O^T accumulates V-stationary: O^T += v_blk^T @ P (one LDW +
one N=512 matmul per k-block instead of v1's four). Softmax
denominators come from ones-weight matmuls into a [1, 512] PSUM row;
normalization broadcasts 1/den across partitions with a K=1 matmul and
multiplies on the DVE. Output is O^T [H, TQ]; the host transposes and
interleaves.

The builder takes reps=N to repeat the whole computation N times inside
one NEFF (used by test.py to amortize dispatch overhead when timing).
"""

import numpy as np
import ml_dtypes

B, T, C, H = 4, 4096, 2048, 128
NCORES = 8
TQ = T // 2              # per-core q rows
NCC = C // 128           # 16 contraction chunks
NSB = TQ // 512          # 4 q superblocks of 512 rows per core
SCALE = float(C) ** -0.5
BF16 = ml_dtypes.bfloat16

_cached = {}


def _build_rep(nc, tc, mybir, rep, xT, wq, wk, wv, cst, out):
    from concourse import tile  # noqa: F401

    f32 = mybir.dt.float32
    bf16 = mybir.dt.bfloat16
    AF = mybir.ActivationFunctionType
    R = f"r{rep}"

    with tc.tile_pool(name=f"sb{R}", bufs=1) as sb, \
         tc.tile_pool(name=f"sbs{R}", bufs=4) as sbs, \
         tc.tile_pool(name=f"p_sb{R}", bufs=3) as p_pool, \
         tc.tile_pool(name=f"o_sb{R}", bufs=2) as o_pool:

        # ---- resident loads: small weight/const DMAs FIRST so the
        # first projection matmuls aren't queued behind the 16.8MB xT.
        # Weights arrive host-prearranged in SBUF layout [128, NCC*H]
        # (contiguous per-partition lines -> full DMA rate) -------------
        w_sb = {}
        for name, h in (("wk", wk), ("wv", wv), ("wq", wq)):
            t = sb.tile([128, NCC * H], bf16, tag=name)
            nc.sync.dma_start(t[:], h.ap())
            w_sb[name] = t

        cst_sb = sb.tile([128, 3 * 128], bf16)
        nc.sync.dma_start(cst_sb[:], cst.ap())
        tri = cst_sb[:, 0:128]
        iden = cst_sb[:, 128:256]
        mpar = cst_sb[:, 256:384]       # ones for p=1, zeros for p=0
        zeros = sb.tile([128, 128], bf16)
        nc.gpsimd.memset(zeros[:], 0.0)
        ones_col = sb.tile([128, 1], bf16)
        nc.gpsimd.memset(ones_col[:], 1.0)
        ones_row = sb.tile([1, 128], f32)
        nc.gpsimd.memset(ones_row[:], 1.0)

        def wchunk(name, cc):
            return w_sb[name][:, cc * H:(cc + 1) * H]

        kT_sb = sb.tile([128, T], bf16)        # [h, t] (position order)
        vT_sb = sb.tile([128, T], bf16)        # [h, t]
        v_sb = sb.tile([128, 32 * H], bf16)    # 32 blocks of [128k, 128h]
        qT_sb = sb.tile([128, TQ], bf16)

        # xT chunks live in their own pool that closes right after the q
        # projection (their last consumer): the NEXT rep's 16.8MB input
        # stream can then DMA during THIS rep's attention phase, which
        # otherwise leaves the DMA engines idle.
        xp_cm = tc.tile_pool(name=f"xp{R}", bufs=1)
        xp = xp_cm.__enter__()
        xc = [xp.tile([128, T], bf16, tag=f"xc{cc}", name=f"xc{cc}{R}")
              for cc in range(NCC)]
        for cc in range(NCC):
            nc.sync.dma_start(xc[cc][:],
                              xT.ap()[128 * cc:128 * (cc + 1), :])

        # ---- projections in three 8-bank passes -----------------------
        # Pass A (DMA-stream-paced): kT first half + the FULL q projection
        # ride in the shadow of the 16.8MB xT stream; pass B (kT second
        # half + vT first half) and pass C (vT second half) then run
        # PE-bound from the resident chunks. This removes q's ~14us from
        # the serial PE path after the stream.
        with tc.tile_pool(name=f"ps_kv{R}", bufs=1, space="PSUM") as pskv:
            psk = [pskv.tile([128, 512], f32, tag=f"psk{g}",
                            name=f"psk{g}{R}")
                   for g in range(4)]
            psq = [pskv.tile([128, 512], f32, tag=f"psq{g}",
                            name=f"psq{g}{R}")
                   for g in range(4)]
            for cc in range(NCC):
                for g in range(4):
                    nc.tensor.matmul(
                        psk[g][:], wchunk("wk", cc),
                        xc[cc][:, 512 * g:512 * (g + 1)],
                        start=(cc == 0), stop=(cc == NCC - 1))
                xcb = xc[cc][:].rearrange("p (nb tb) -> p nb tb", tb=128)
                for g in range(4):
                    nc.tensor.matmul(
                        psq[g][:], wchunk("wq", cc),
                        xcb[:, 8 * g:8 * g + 8:2, :],
                        start=(cc == 0), stop=(cc == NCC - 1))
            for g in range(4):
                nc.scalar.copy(kT_sb[:, 512 * g:512 * (g + 1)], psk[g][:])
                nc.vector.tensor_copy(qT_sb[:, 512 * g:512 * (g + 1)],
                                      psq[g][:])

            # Pass B: kT half 1 + vT half 0 (tags reuse pass A's banks)
            psk2 = [pskv.tile([128, 512], f32, tag=f"psk{g}",
                             name=f"psk2{g}{R}")
                    for g in range(4)]
            psv = [pskv.tile([128, 512], f32, tag=f"psq{g}",
                            name=f"psv{g}{R}")
                   for g in range(4)]
            for cc in range(NCC):
                for g in range(4):
                    nc.tensor.matmul(
                        psk2[g][:], wchunk("wk", cc),
                        xc[cc][:, 2048 + 512 * g:2048 + 512 * (g + 1)],
                        start=(cc == 0), stop=(cc == NCC - 1))
                for g in range(4):
                    nc.tensor.matmul(
                        psv[g][:], wchunk("wv", cc),
                        xc[cc][:, 512 * g:512 * (g + 1)],
                        start=(cc == 0), stop=(cc == NCC - 1))
            for g in range(4):
                nc.scalar.copy(kT_sb[:, 2048 + 512 * g:2048 + 512 * (g + 1)],
                               psk2[g][:])
                nc.vector.tensor_copy(vT_sb[:, 512 * g:512 * (g + 1)],
                                      psv[g][:])

            # Pass C: vT half 1
            psv2 = [pskv.tile([128, 512], f32, tag=f"psk{g}",
                             name=f"psv2{g}{R}")
                    for g in range(4)]
            for cc in range(NCC):
                for g in range(4):
                    nc.tensor.matmul(
                        psv2[g][:], wchunk("wv", cc),
                        xc[cc][:, 2048 + 512 * g:2048 + 512 * (g + 1)],
                        start=(cc == 0), stop=(cc == NCC - 1))
            for g in range(4):
                nc.vector.tensor_copy(
                    vT_sb[:, 2048 + 512 * g:2048 + 512 * (g + 1)], psv2[g][:])
        xp_cm.__exit__(None, None, None)

        # ---- v: [h, t] -> [k, h] blocks via PE transpose --------------
        with tc.tile_pool(name=f"ps_tr{R}", bufs=2, space="PSUM") as pstr:
            for j in range(32):
                pst = pstr.tile([128, 128], bf16)
                nc.tensor.transpose(pst[:], vT_sb[:, 128 * j:128 * (j + 1)],
                                    iden)
                nc.vector.tensor_copy(v_sb[:, H * j:H * (j + 1)], pst[:])

        def v_blk(j):
            return v_sb[:, H * j:H * (j + 1)]

        def kt_blk(j):
            return kT_sb[:, 128 * j:128 * (j + 1)]

        # ---- attention -----------------------------------------------
        with tc.tile_pool(name=f"ps_s{R}", bufs=3, space="PSUM") as pssp, \
             tc.tile_pool(name=f"ps_o{R}", bufs=1, space="PSUM") as psop, \
             tc.tile_pool(name=f"ps_d{R}", bufs=1, space="PSUM") as psdp:

            for s in range(NSB):
                npair = 4 * s + 4
                o_ps = psop.tile([128, 512], f32, tag="o")
                den_t = psdp.tile([128, 512], f32, tag="d")
                den_ps = den_t[0:1, :]
                for u in range(npair):
                    # diagonal-band pairs only touch the upper q columns:
                    # band position dd = 2*u_rel+half is zero for all
                    # chunks c with 2c+1 < dd, so pair u_rel needs
                    # q columns >= 128*u_rel only.
                    u_rel = u - 4 * s
                    qoff = 128 * u_rel if u_rel > 0 else 0
                    nq = 512 - qoff
                    # halves live at bank-aligned offsets 0 and 512
                    pss = pssp.tile([128, 1024], f32, tag="s")
                    for half in range(2):
                        j = 2 * u + half
                        nc.tensor.matmul(
                            pss[:, 512 * half:512 * half + nq],
                            kt_blk(j),
                            qT_sb[:, 512 * s + qoff:512 * (s + 1)],
                            start=True, stop=True, skip_group_check=True)
                    P = p_pool.tile([128, 1024], bf16)
                    if qoff == 0:
                        nc.scalar.activation(P[:], pss[:], AF.Exp,
                                             scale=SCALE)
                    else:
                        for half in range(2):
                            nc.scalar.activation(
                                P[:, 512 * half:512 * half + nq],
                                pss[:, 512 * half:512 * half + nq],
                                AF.Exp, scale=SCALE)
                    if u_rel >= 0:
                        for half in range(2):
                            dd = 2 * u_rel + half
                            for c in range(qoff // 128, 4):
                                if dd < 2 * c:
                                    continue          # fully allowed
                                pc = P[:, 512 * half + 128 * c - qoff:
                                       512 * half + 128 * (c + 1) - qoff]
                                m = tri if dd == 2 * c else mpar
                                nc.vector.tensor_mul(pc, pc, m)
                    # both AV halves first, then both den halves: the
                    # ones_col weight is loaded once per pair, not twice
                    for half in range(2):
                        j = 2 * u + half
                        nc.tensor.matmul(
                            o_ps[:, qoff:512], v_blk(j),
                            P[:, 512 * half:512 * half + nq],
                            start=(u == 0 and half == 0),
                            stop=(u == npair - 1 and half == 1),
                            skip_group_check=True)
                    for half in range(2):
                        nc.tensor.matmul(
                            den_t[0:1, qoff:512], ones_col[:],
                            P[:, 512 * half:512 * half + nq],
                            start=(u == 0 and half == 0),
                            stop=(u == npair - 1 and half == 1),
                            skip_group_check=True)

                # normalize: bcast 1/den across partitions, multiply, store
                rec = sbs.tile([1, 512], f32, tag="rec")
                nc.vector.reciprocal(rec[:], den_ps)
                bc_ps = pssp.tile([128, 1024], f32, tag="s")
                nc.tensor.matmul(bc_ps[:, 0:512], ones_row[:], rec[:],
                                 start=True, stop=True, skip_group_check=True)
                bc_sb = sbs.tile([128, 512], f32, tag="bc")
                nc.scalar.copy(bc_sb[:], bc_ps[:, 0:512])
                osb = o_pool.tile([128, 512], f32, tag="osb")
                nc.vector.tensor_mul(osb[:], o_ps[:], bc_sb[:])
                nc.sync.dma_start(out.ap()[:, 512 * s:512 * (s + 1)], osb[:])


def _build_nc(reps=1):
    import concourse.bacc as bacc
    import concourse.mybir as mybir
    from concourse import tile

    f32 = mybir.dt.float32
    bf16 = mybir.dt.bfloat16

    nc = bacc.Bacc("TRN2", target_bir_lowering=False, debug=False,
                   num_devices=NCORES)

    xT = nc.declare_dram_parameter("xT", [C, T], bf16, isOutput=False)
    # weights host-prearranged to [128, NCC*H]: row p, col n*H+h = W[n*128+p, h]
    wq = nc.declare_dram_parameter("Wq", [128, NCC * H], bf16, isOutput=False)
    wk = nc.declare_dram_parameter("Wk", [128, NCC * H], bf16, isOutput=False)
    wv = nc.declare_dram_parameter("Wv", [128, NCC * H], bf16, isOutput=False)
    # consts: [tri | identity | parity-mask]
    cst = nc.declare_dram_parameter("consts", [128, 3 * 128], bf16,
                                    isOutput=False)
    out = nc.declare_dram_parameter("outT", [H, TQ], f32, isOutput=True)

    with tile.TileContext(nc) as tc:
        for rep in range(reps):
            _build_rep(nc, tc, mybir, rep, xT, wq, wk, wv, cst, out)

    nc.finalize()
    return nc


# p=1 pair swap: position 2i <- global 2i+1, position 2i+1 <- global 2i
_PERM1 = np.arange(32).reshape(16, 2)[:, ::-1].reshape(32)


def _build_consts(p):
    kk = np.arange(128)[:, None]
    tt = np.arange(128)[None, :]
    tri = (kk <= tt).astype(np.float32)
    iden = np.eye(128, dtype=np.float32)
    mpar = np.full((128, 128), float(p), np.float32)
    M = np.concatenate([tri, iden, mpar], axis=1)
    return np.ascontiguousarray(M.astype(BF16))


def _get_nc():
    if "nc" not in _cached:
        _cached["nc"] = _build_nc()
    return _cached["nc"]


def _prep_in_maps(x, Wq, Wk, Wv):
    def _prew(w):
        # [C, H] -> [128, NCC*H] with row p, col n*H+h = W[n*128+p, h]
        a = np.asarray(w).astype(BF16).reshape(NCC, 128, H)
        return np.ascontiguousarray(a.transpose(1, 0, 2).reshape(128, NCC * H))
    w16 = {n: _prew(w) for n, w in (("Wq", Wq), ("Wk", Wk), ("Wv", Wv))}
    consts = {p: _build_consts(p) for p in (0, 1)}
    xTb = {}
    for b in range(B):
        xt = np.asarray(x[b]).astype(BF16).T          # [C, T]
        xTb[b, 0] = np.ascontiguousarray(xt)
        xTb[b, 1] = np.ascontiguousarray(
            xt.reshape(C, 32, 128)[:, _PERM1, :].reshape(C, T))
    in_maps = []
    for c in range(NCORES):
        b, p = divmod(c, 2)
        in_maps.append({"xT": xTb[b, p], "consts": consts[p], **w16})
    return in_maps


def _gather_out(results):
    out = np.empty((B, T, H), np.float32)
    for c in range(NCORES):
        b, p = divmod(c, 2)
        oT = results[c]["outT"]          # [H, TQ], own rows ascending
        out[b].reshape(T // 128, 128, H)[p::2] = \
            np.ascontiguousarray(oT.T).reshape(TQ // 128, 128, H)
    return out


def kernel(x, Wq, Wk, Wv):
    from concourse.bass_utils import run_bass_kernel_spmd

    nc = _get_nc()
    in_maps = _prep_in_maps(x, Wq, Wk, Wv)
    res = run_bass_kernel_spmd(nc, in_maps, list(range(NCORES)))
    return _gather_out(res.results)


# revision 21
# speedup vs baseline: 1.0655x; 1.0655x over previous
"""Trainium2 Bass kernel v2: single-head causal attention
(B=4, T=4096, C=2048, H=128), zero collectives.

    q = x @ Wq; k = x @ Wk; v = x @ Wv        (per batch element)
    out = softmax(causal(q k^T * C**-0.5)) @ v

Sharding: two cores per batch element (8 cores, B=4). Within a batch the
4096 q rows are split between the pair by 128-row-block parity (core p
owns blocks p, p+2, ...), which balances the causal work. Unlike v1,
each core projects k/v for the FULL 4096 rows from the full xT (the
extra ~4.3 GFLOP of redundant projection removes both AllGathers and all
cross-core synchronization).

The host permutes xT's 128-column blocks into "pair-local" order (own
parity block i at position 2i, the sibling block at 2i+1), so one
parity-free program serves all cores: q projects from even positions,
the causal band rule in position terms is dd==2c -> tri mask,
dd==2c+1 -> per-core mask (ones for p=1, zeros for p=0), dd>2c+1 ->
zeros.

Projections are weight-stationary (Wk/Wv/Wq chunks as PE weights, xT
chunks as the N=512 mover): kT and vT come out in [H, T] layout with
only 16 LDWEIGHTS each. V is then flipped to [k, h] blocks with 32
PE-mode transposes. Attention runs scores-transposed: S^T = kT_blk^T @
qT, P = exp(S^T * scale) on ACT, masking on the DVE (diagonal band
only), then O^T accumulates V-stationary: O^T += v_blk^T @ P (one LDW +
one N=512 matmul per k-block instead of v1's four). Softmax
denominators come from ones-weight matmuls into a [1, 512] PSUM row;
normalization broadcasts 1/den across partitions with a K=1 matmul and
multiplies on the DVE. Output is O^T [H, TQ]; the host transposes and
interleaves.

The builder takes reps=N to repeat the whole computation N times inside
one NEFF (used by test.py to amortize dispatch overhead when timing).
"""

import numpy as np
import ml_dtypes

B, T, C, H = 4, 4096, 2048, 128
NCORES = 8
TQ = T // 2              # per-core q rows
NCC = C // 128           # 16 contraction chunks
NSB = TQ // 512          # 4 q superblocks of 512 rows per core
SCALE = float(C) ** -0.5
BF16 = ml_dtypes.bfloat16

_cached = {}


def _build_rep(nc, tc, mybir, rep, xT, wq, wk, wv, cst, out):
    from concourse import tile  # noqa: F401

    f32 = mybir.dt.float32
    bf16 = mybir.dt.bfloat16
    AF = mybir.ActivationFunctionType
    R = f"r{rep}"

    with tc.tile_pool(name=f"sb{R}", bufs=1) as sb, \
         tc.tile_pool(name=f"sbs{R}", bufs=4) as sbs, \
         tc.tile_pool(name=f"p_sb{R}", bufs=3) as p_pool, \
         tc.tile_pool(name=f"o_sb{R}", bufs=2) as o_pool:

        # ---- resident loads: small weight/const DMAs FIRST so the
        # first projection matmuls aren't queued behind the 16.8MB xT.
        # Weights arrive host-prearranged in SBUF layout [128, NCC*H]
        # (contiguous per-partition lines -> full DMA rate) -------------
        w_sb = {}
        for name, h in (("wk", wk), ("wv", wv), ("wq", wq)):
            t = sb.tile([128, NCC * H], bf16, tag=name)
            nc.sync.dma_start(t[:], h.ap())
            w_sb[name] = t

        cst_sb = sb.tile([128, 3 * 128], bf16)
        nc.sync.dma_start(cst_sb[:], cst.ap())
        tri = cst_sb[:, 0:128]
        iden = cst_sb[:, 128:256]
        mpar = cst_sb[:, 256:384]       # ones for p=1, zeros for p=0
        zeros = sb.tile([128, 128], bf16)
        nc.gpsimd.memset(zeros[:], 0.0)
        ones_col = sb.tile([128, 1], bf16)
        nc.gpsimd.memset(ones_col[:], 1.0)
        ones_row = sb.tile([1, 128], f32)
        nc.gpsimd.memset(ones_row[:], 1.0)

        def wchunk(name, cc):
            return w_sb[name][:, cc * H:(cc + 1) * H]

        kT_sb = sb.tile([128, T], bf16)        # [h, t] (position order)
        vT_sb = sb.tile([128, T], bf16)        # [h, t]
        v_sb = sb.tile([128, 32 * H], bf16)    # 32 blocks of [128k, 128h]
        qT_sb = sb.tile([128, TQ], bf16)

        # xT chunks live in their own pool that closes right after the q
        # projection (their last consumer): the NEXT rep's 16.8MB input
        # stream can then DMA during THIS rep's attention phase, which
        # otherwise leaves the DMA engines idle.
        xp_cm = tc.tile_pool(name=f"xp{R}", bufs=1)
        xp = xp_cm.__enter__()
        xc = [xp.tile([128, T], bf16, tag=f"xc{cc}", name=f"xc{cc}{R}")
              for cc in range(NCC)]
        for cc in range(NCC):
            nc.sync.dma_start(xc[cc][:],
                              xT.ap()[128 * cc:128 * (cc + 1), :])

        # ---- fused k^T / v^T projection over full T -------------------
        with tc.tile_pool(name=f"ps_kv{R}", bufs=1, space="PSUM") as pskv:
            for th in range(2):               # T halves to fit PSUM
                t0 = 2048 * th
                psk = [pskv.tile([128, 512], f32, tag=f"psk{g}",
                                name=f"psk{g}{R}")
                       for g in range(4)]
                psv = [pskv.tile([128, 512], f32, tag=f"psv{g}",
                                name=f"psv{g}{R}")
                       for g in range(4)]
                for cc in range(NCC):
                    for g in range(4):
                        nc.tensor.matmul(
                            psk[g][:], wchunk("wk", cc),
                            xc[cc][:, t0 + 512 * g:t0 + 512 * (g + 1)],
                            start=(cc == 0), stop=(cc == NCC - 1))
                    for g in range(4):
                        nc.tensor.matmul(
                            psv[g][:], wchunk("wv", cc),
                            xc[cc][:, t0 + 512 * g:t0 + 512 * (g + 1)],
                            start=(cc == 0), stop=(cc == NCC - 1))
                for g in range(4):
                    nc.scalar.copy(kT_sb[:, t0 + 512 * g:t0 + 512 * (g + 1)],
                                   psk[g][:])
                    nc.vector.tensor_copy(
                        vT_sb[:, t0 + 512 * g:t0 + 512 * (g + 1)], psv[g][:])

        # ---- v: [h, t] -> [k, h] blocks via PE transpose --------------
        with tc.tile_pool(name=f"ps_tr{R}", bufs=2, space="PSUM") as pstr:
            for j in range(32):
                pst = pstr.tile([128, 128], bf16)
                nc.tensor.transpose(pst[:], vT_sb[:, 128 * j:128 * (j + 1)],
                                    iden)
                nc.vector.tensor_copy(v_sb[:, H * j:H * (j + 1)], pst[:])

        def v_blk(j):
            return v_sb[:, H * j:H * (j + 1)]

        def kt_blk(j):
            return kT_sb[:, 128 * j:128 * (j + 1)]

        # ---- q^T projection (own rows = even positions) ---------------
        with tc.tile_pool(name=f"ps_q{R}", bufs=1, space="PSUM") as psqp:
            psq = [psqp.tile([128, 512], f32, tag=f"psq{g}",
                             name=f"psq{g}{R}") for g in range(4)]
            for cc in range(NCC):
                for g in range(4):
                    xcb = xc[cc][:].rearrange("p (nb tb) -> p nb tb",
                                              tb=128)
                    nc.tensor.matmul(
                        psq[g][:], wchunk("wq", cc),
                        xcb[:, 8 * g:8 * g + 8:2, :],
                        start=(cc == 0), stop=(cc == NCC - 1))
            for g in range(4):
                nc.vector.tensor_copy(qT_sb[:, 512 * g:512 * (g + 1)],
                                      psq[g][:])
        xp_cm.__exit__(None, None, None)

        # ---- attention -----------------------------------------------
        with tc.tile_pool(name=f"ps_s{R}", bufs=3, space="PSUM") as pssp, \
             tc.tile_pool(name=f"ps_o{R}", bufs=1, space="PSUM") as psop, \
             tc.tile_pool(name=f"ps_d{R}", bufs=1, space="PSUM") as psdp:

            for s in range(NSB):
                npair = 4 * s + 4
                o_ps = psop.tile([128, 512], f32, tag="o")
                den_t = psdp.tile([128, 512], f32, tag="d")
                den_ps = den_t[0:1, :]
                for u in range(npair):
                    # diagonal-band pairs only touch the upper q columns:
                    # band position dd = 2*u_rel+half is zero for all
                    # chunks c with 2c+1 < dd, so pair u_rel needs
                    # q columns >= 128*u_rel only.
                    u_rel = u - 4 * s
                    qoff = 128 * u_rel if u_rel > 0 else 0
                    nq = 512 - qoff
                    # halves live at bank-aligned offsets 0 and 512
                    pss = pssp.tile([128, 1024], f32, tag="s")
                    for half in range(2):
                        j = 2 * u + half
                        nc.tensor.matmul(
                            pss[:, 512 * half:512 * half + nq],
                            kt_blk(j),
                            qT_sb[:, 512 * s + qoff:512 * (s + 1)],
                            start=True, stop=True, skip_group_check=True)
                    P = p_pool.tile([128, 1024], bf16)
                    if qoff == 0:
                        nc.scalar.activation(P[:], pss[:], AF.Exp,
                                             scale=SCALE)
                    else:
                        for half in range(2):
                            nc.scalar.activation(
                                P[:, 512 * half:512 * half + nq],
                                pss[:, 512 * half:512 * half + nq],
                                AF.Exp, scale=SCALE)
                    if u_rel >= 0:
                        for half in range(2):
                            dd = 2 * u_rel + half
                            for c in range(qoff // 128, 4):
                                if dd < 2 * c:
                                    continue          # fully allowed
                                pc = P[:, 512 * half + 128 * c - qoff:
                                       512 * half + 128 * (c + 1) - qoff]
                                m = tri if dd == 2 * c else mpar
                                nc.vector.tensor_mul(pc, pc, m)
                    # both AV halves first, then both den halves: the
                    # ones_col weight is loaded once per pair, not twice
                    for half in range(2):
                        j = 2 * u + half
                        nc.tensor.matmul(
                            o_ps[:, qoff:512], v_blk(j),
                            P[:, 512 * half:512 * half + nq],
                            start=(u == 0 and half == 0),
                            stop=(u == npair - 1 and half == 1),
                            skip_group_check=True)
                    for half in range(2):
                        nc.tensor.matmul(
                            den_t[0:1, qoff:512], ones_col[:],
                            P[:, 512 * half:512 * half + nq],
                            start=(u == 0 and half == 0),
                            stop=(u == npair - 1 and half == 1),
                            skip_group_check=True)

                # normalize: bcast 1/den across partitions, multiply, store
                rec = sbs.tile([1, 512], f32, tag="rec")
                nc.vector.reciprocal(rec[:], den_ps)
                bc_ps = pssp.tile([128, 1024], f32, tag="s")
                nc.tensor.matmul(bc_ps[:, 0:512], ones_row[:], rec[:],
                                 start=True, stop=True, skip_group_check=True)
                bc_sb = sbs.tile([128, 512], f32, tag="bc")
                nc.scalar.copy(bc_sb[:], bc_ps[:, 0:512])
                osb = o_pool.tile([128, 512], f32, tag="osb")
                nc.vector.tensor_mul(osb[:], o_ps[:], bc_sb[:])
                nc.sync.dma_start(out.ap()[:, 512 * s:512 * (s + 1)], osb[:])


def _build_nc(reps=1):
    import concourse.bacc as bacc
    import concourse.mybir as mybir
    from concourse import tile

    f32 = mybir.dt.float32
    bf16 = mybir.dt.bfloat16

    nc = bacc.Bacc("TRN2", target_bir_lowering=False, debug=False,
                   num_devices=NCORES)

    xT = nc.declare_dram_parameter("xT", [C, T], bf16, isOutput=False)
    # weights host-prearranged to [128, NCC*H]: row p, col n*H+h = W[n*128+p, h]
    wq = nc.declare_dram_parameter("Wq", [128, NCC * H], bf16, isOutput=False)
    wk = nc.declare_dram_parameter("Wk", [128, NCC * H], bf16, isOutput=False)
    wv = nc.declare_dram_parameter("Wv", [128, NCC * H], bf16, isOutput=False)
    # consts: [tri | identity | parity-mask]
    cst = nc.declare_dram_parameter("consts", [128, 3 * 128], bf16,
                                    isOutput=False)
    out = nc.declare_dram_parameter("outT", [H, TQ], f32, isOutput=True)

    with tile.TileContext(nc) as tc:
        for rep in range(reps):
            _build_rep(nc, tc, mybir, rep, xT, wq, wk, wv, cst, out)

    nc.finalize()
    return nc


# p=1 pair swap: position 2i <- global 2i+1, position 2i+1 <- global 2i
_PERM1 = np.arange(32).reshape(16, 2)[:, ::-1].reshape(32)


def _build_consts(p):
    kk = np.arange(128)[:, None]
    tt = np.arange(128)[None, :]
    tri = (kk <= tt).astype(np.float32)
    iden = np.eye(128, dtype=np.float32)
    mpar = np.full((128, 128), float(p), np.float32)
    M = np.concatenate([tri, iden, mpar], axis=1)
    return np.ascontiguousarray(M.astype(BF16))


def _get_nc():
    if "nc" not in _cached:
        _cached["nc"] = _build_nc()
    return _cached["nc"]


def _prep_in_maps(x, Wq, Wk, Wv):
    def _prew(w):
        # [C, H] -> [128, NCC*H] with row p, col n*H+h = W[n*128+p, h]
        a = np.asarray(w).astype(BF16).reshape(NCC, 128, H)
        return np.ascontiguousarray(a.transpose(1, 0, 2).reshape(128, NCC * H))
    w16 = {n: _prew(w) for n, w in (("Wq", Wq), ("Wk", Wk), ("Wv", Wv))}
    consts = {p: _build_consts(p) for p in (0, 1)}
    xTb = {}
    for b in range(B):
        xt = np.asarray(x[b]).astype(BF16).T          # [C, T]
        xTb[b, 0] = np.ascontiguousarray(xt)
        xTb[b, 1] = np.ascontiguousarray(
            xt.reshape(C, 32, 128)[:, _PERM1, :].reshape(C, T))
    in_maps = []
    for c in range(NCORES):
        b, p = divmod(c, 2)
        in_maps.append({"xT": xTb[b, p], "consts": consts[p], **w16})
    return in_maps


def _gather_out(results):
    out = np.empty((B, T, H), np.float32)
    for c in range(NCORES):
        b, p = divmod(c, 2)
        oT = results[c]["outT"]          # [H, TQ], own rows ascending
        out[b].reshape(T // 128, 128, H)[p::2] = \
            np.ascontiguousarray(oT.T).reshape(TQ // 128, 128, H)
    return out


def kernel(x, Wq, Wk, Wv):
    from concourse.bass_utils import run_bass_kernel_spmd

    nc = _get_nc()
    in_maps = _prep_in_maps(x, Wq, Wk, Wv)
    res = run_bass_kernel_spmd(nc, in_maps, list(range(NCORES)))
    return _gather_out(res.results)
